# revision 20
# baseline (speedup 1.0000x reference)
"""ColBERT MaxSim contrastive loss on 8 Trainium2 NeuronCores.

scores[b, c] = (1/q_len[b]) * sum_n max_s <q[b, n, :], d[c, s, :]>
loss = CE(scores / T, labels=arange(B)), mean reduction.

Sharding: data-parallel over the *doc* batch dim (columns of the score
matrix). Each core holds the full query set plus its 8-doc shard,
computes its (64, 8) score block, and the host performs the final
gather + tiny 64x64 CE reduction.

v2 design (trace-driven):
  * Host pre-transposes and fp16-casts q and d into [D, token] layout,
    so the device does plain contiguous DMAs (2-4 KB per partition) and
    starts the first matmul within a few us -- the v1 on-device
    load+cast+xbar-transpose prologue cost 28 us of PE idle.
  * Per (query-group g, doc) set: two fp16 matmuls -> [128, 1024] PSUM
    tile (4 rotating slots = all 8 banks). PE streams 512 cols / 215 ns.
  * PSUM max-drain, the v1 bottleneck (ACT 117us + DVE 108us busy),
    is split across both engines with single-instruction fused ops:
      - DVE docs: tensor_tensor_reduce(max, max) reading the two PSUM
        halves via both operand ports (2 elem/cyc) -> exact max in one
        ~557 ns op.
      - ACT docs: activation(Exp, scale=a, bias=-0.9a, accum_out=sum)
        drains 1024 elems at 1 elem/cyc; a single Log pass at the end
        turns the per-doc sums into smooth-maxes:
            max_s x ~= 0.9 + ln(sum_s e^{a(x_s-0.9)}) / a,  a = 512.
        Upward bias <= ln(1024)/a ~ 0.0135 worst case, ~0.002 typical,
        and mostly common-mode across score columns (cancels in
        softmax); measured end-to-end loss rel-err 6e-5.
  * A tiny selector matmul sums the 32 token-maxes per query:
    out[4, 128] = sel.T @ maxes. Host unscrambles + CE.
"""

import json

import numpy as np

import concourse.bass as bass
import concourse.mybir as mybir
import concourse.tile as tile
from concourse.bass_utils import run_bass_kernel_spmd

B = 64          # queries (= docs, contrastive batch)
NQ = 32         # tokens per query
ND = 1024       # tokens per doc
D = 128         # embedding dim
NCORES = 8
CL = B // NCORES  # docs per core
TEMPERATURE = 0.02
NORMALIZE_SCORES = True

F32 = mybir.dt.float32
F16 = mybir.dt.float16

NG = (B * NQ) // 128        # 16 query groups of 4 queries
NSETS = NG * CL             # 128 (query group, doc) sets

ALPHA = 512.0               # smooth-max sharpness
CENTER = 0.9                # exp recentering (sims in [0.55, 0.90])

# Per-doc drain engine assignment. DVE can read only ONE operand from
# PSUM per instruction (NCC_IBVF027), so both drain engines run at
# ~1 elem/cyc. Measured steady-state costs: DVE reduce_max 1215ns/doc
# (1035ns/doc when two adjacent PSUM slots batch into one [128,2,1024]
# reduce), ACT exp-accum 1113+283(READ_ACC) ns/doc. A 5/3 split with
# two batched DVE pairs balances at ~5.4us vs ~4.6us per group.
DVE_DOCS = (0, 1, 2, 4, 5)  # exact reduce_max on DVE; (0,1) and (4,5)
DVE_PAIRS = ((0, 1), (4, 5))  # batched: adjacent PSUM slots, one reduce
DVE_SINGLES = (2,)
ACT_DOCS = (3, 6, 7)        # smooth-max exp-accum on ACT
MAX_EXP = len(ACT_DOCS)     # exp docs per group (sums tile width)
ACT_SLOT = {c: i for i, c in enumerate(ACT_DOCS)}


def _split_waits_json(bir_bytes: bytes) -> bytes:
    """Walrus in this toolchain rejects >1 sem-wait per instruction on the
    Tile end-of-kernel drain; split extra waits onto preceding Drains."""
    bir = json.loads(bir_bytes)
    for f in bir["functions"]:
        for blk in f["blocks"]:
            fixed = []
            for ins in blk["instructions"]:
                si = ins.get("sync_info") or {}
                waits = si.get("on_wait") or []
                if len(waits) > 1:
                    for i, w in enumerate(waits[:-1]):
                        fixed.append({
                            "debug": ins.get("debug", 0),
                            "engine": ins["engine"],
                            "ins": [],
                            "is_reset_sema": False,
                            "name": f'{ins["name"]}-wsplit{i}',
                            "opcode": "Drain",
                            "outs": [],
                            "sync_info": {"on_update": [], "on_wait": [w]},
                        })
                    si["on_wait"] = waits[-1:]
                    ins["sync_info"] = si
                fixed.append(ins)
            blk["instructions"] = fixed
    return json.dumps(bir).encode()


def _patch_nc(nc):
    orig = nc.to_json_bytes

    def patched(*a, **k):
        return _split_waits_json(orig(*a, **k))

    nc.to_json_bytes = patched
    return nc


def build_nc():
    """Build the per-core Bass program (SPMD: every core runs this; only
    the data in its "dT" shard differs)."""
    nc = bass.Bass("TRN2", target_bir_lowering=False, debug=False,
                   num_devices=NCORES)
    qT_dram = nc.dram_tensor("qT", [D, B * NQ], F16, kind="ExternalInput").ap()
    dT_dram = nc.dram_tensor("dT", [D, CL * ND], F16,
                             kind="ExternalInput").ap()
    sel_dram = nc.dram_tensor("sel", [128, 4], F16, kind="ExternalInput").ap()
    out_dram = nc.dram_tensor("out", [4, NSETS], F32, kind="ExternalOutput").ap()
    sums_dram = nc.dram_tensor("sums", [128, NG * MAX_EXP], F32,
                               kind="ExternalOutput").ap()

    EXP = mybir.ActivationFunctionType.Exp

    with tile.TileContext(nc) as tc:
        with (
            tc.tile_pool(name="prep", bufs=1) as prep,
            tc.tile_pool(name="eo", bufs=2) as eo_pool,
            tc.tile_pool(name="mm", bufs=2, space="PSUM") as psum_pool,
        ):
            # ---- loads: host-pretransposed fp16 [D, token] tensors.
            # Split so the g=0 chain (qT group 0 + docs 0-1) lands first,
            # alternating issue queues (sync/scalar are the HWDGE
            # engines); each chunk is a contiguous 4 KB/partition. ----
            qT0 = prep.tile([128, 128], F16, tag="qT0", name="qT0")
            qTr = prep.tile([128, (NG - 1) * 128], F16, tag="qTr",
                            name="qTr")
            nc.sync.dma_start(qT0[:], qT_dram[:, 0:128])
            nc.scalar.dma_start(qTr[:], qT_dram[:, 128:NG * 128])
            dT = []   # 4 tiles of 2 docs each
            for p in range(CL // 2):
                t = prep.tile([128, 2 * ND], F16, tag=f"dT{p}",
                              name=f"dT{p}")
                eng = nc.sync if p % 2 == 0 else nc.scalar
                eng.dma_start(t[:], dT_dram[:, 2 * p * ND:(2 * p + 2) * ND])
                dT.append(t)
            sel = prep.tile([128, 4], F16)
            nc.sync.dma_start(sel[:], sel_dram)

            maxes = prep.tile([128, NSETS], F16)
            sums = prep.tile([128, NG * MAX_EXP], F32)
            # exp-doc columns of `maxes` are never written on-device (the
            # host takes them from `sums`); zero them so the selector
            # matmul can't see garbage/NaN.
            nc.vector.memset(maxes[:], 0.0)
            # warm the ACT exp table-set (~2.7us) during the DMA window
            # instead of on the first real exp-drain; exp_bias doubles as
            # the (AP-only) bias operand of the real exp-drains
            warm = prep.tile([128, 1], F32)
            nc.vector.memset(warm[:], 0.0)
            exp_bias = prep.tile([128, 1], F32)
            nc.vector.memset(exp_bias[:], -ALPHA * CENTER)
            nc.scalar.activation(warm[:], warm[:], EXP)

            # ---- main loop: 16 query groups x 8 docs, two docs per
            # [128, 2048] PSUM tile (2 tiles = all 8 banks) ----
            for g in range(NG):
                lhs = qT0 if g == 0 else qTr[:, bass.ts(g - 1, 128)]
                for pair in range(CL // 2):
                    d0, d1 = 2 * pair, 2 * pair + 1
                    idx = g * CL + d0
                    pa = psum_pool.tile([128, 2 * ND], F32, tag="pa",
                                        name="pa")
                    for half, doc in ((0, d0), (1, d1)):
                        rhs = dT[pair][:, half * ND:(half + 1) * ND]
                        po = half * ND
                        nc.tensor.matmul(pa[:, po:po + 512], lhs,
                                         rhs[:, 0:512], start=True,
                                         stop=True)
                        nc.tensor.matmul(pa[:, po + 512:po + 1024], lhs,
                                         rhs[:, 512:1024], start=True,
                                         stop=True)
                    if (d0, d1) in DVE_PAIRS:
                        # one batched reduce drains both docs
                        nc.vector.reduce_max(
                            maxes[:, idx:idx + 2],
                            pa[:].rearrange("p (s f) -> p s f", s=2),
                            axis=mybir.AxisListType.X)
                        continue
                    for half, doc in ((0, d0), (1, d1)):
                        sl = pa[:, half * ND:(half + 1) * ND]
                        if doc in DVE_SINGLES:
                            nc.vector.reduce_max(
                                maxes[:, idx + half:idx + half + 1], sl,
                                axis=mybir.AxisListType.X)
                        else:
                            # smooth-max: ACT exp-drain with fused sum
                            j = g * MAX_EXP + ACT_SLOT[doc]
                            eo = eo_pool.tile([128, ND], F32, tag="eo",
                                              name="eo")
                            nc.scalar.activation(
                                eo[:], sl, EXP,
                                bias=exp_bias[:], scale=ALPHA,
                                accum_out=sums[:, j:j + 1])
                if g == NG // 2 - 1:
                    # ship the first half of the exp sums early
                    nc.sync.dma_start(
                        sums_dram[:, 0:NG * MAX_EXP // 2],
                        sums[:, 0:NG * MAX_EXP // 2])

            # rest of the exp-doc sums; host computes the ln
            nc.sync.dma_start(sums_dram[:, NG * MAX_EXP // 2:],
                              sums[:, NG * MAX_EXP // 2:])

            # ---- sum the 32 tokens of each query: out[4, NSETS] ----
            sel_ps = psum_pool.tile([4, NSETS], F32, tag="pa", name="selps")
            nc.tensor.matmul(sel_ps[:], sel[:], maxes[:], start=True,
                             stop=True)
            out_sb = prep.tile([4, NSETS], F32)
            nc.vector.tensor_copy(out_sb[:], sel_ps[:])
            nc.sync.dma_start(out_dram, out_sb[:])

    nc.finalize()
    return _patch_nc(nc)


_NC = None


def _get_nc():
    global _NC
    if _NC is None:
        _NC = build_nc()
    return _NC


def make_sel():
    # sel[p, m] = 1 iff token-partition p belongs to query m of its group
    sel = np.zeros((128, 4), np.float16)
    for m in range(4):
        sel[NQ * m:NQ * (m + 1), m] = 1.0
    return sel


def make_inmaps(q, d):
    """Host prep: [D, token] fp16 layouts + per-core doc shards."""
    q = np.asarray(q, dtype=np.float32)
    d = np.asarray(d, dtype=np.float32)
    qT = np.ascontiguousarray(
        q.reshape(B * NQ, D).T.astype(np.float16))          # [128, 2048]
    dT_full = d.transpose(2, 0, 1).astype(np.float16)       # [128, 64, 1024]
    sel = make_sel()
    return [
        {"qT": qT,
         "dT": np.ascontiguousarray(
             dT_full[:, CL * k:CL * (k + 1)]).reshape(D, CL * ND),
         "sel": sel}
        for k in range(NCORES)
    ]


def assemble_loss(outs, sums, q):
    """Host tail: per-core [4, NSETS] + raw exp-sums -> scores -> CE.

    out[m, g*8+c] (DVE docs) = sum over the 32 tokens of query 4g+m of
    the exact token-max. For ACT exp docs the device ships S = sum_s
    e^{a(sim-0.9)} per (token-partition, g, e) and the host applies
    max ~= 0.9 + ln(S)/a and the 32-token sum itself."""
    scores = np.zeros((B, B), np.float64)
    for k in range(NCORES):
        blk = np.asarray(outs[k], np.float64).reshape(4, NG, CL)
        # token-sums of ln(S): [128, NG, MAX_EXP] -> [4, NG, MAX_EXP]
        lnS = np.log(np.asarray(sums[k], np.float64)).reshape(
            4, NQ, NG, MAX_EXP).sum(axis=1)
        for g in range(NG):
            for m in range(4):
                for c in range(CL):
                    if c in DVE_DOCS:
                        v = blk[m, g, c]
                    else:
                        v = NQ * CENTER + lnS[m, g, ACT_SLOT[c]] / ALPHA
                    scores[4 * g + m, CL * k + c] = v
    if NORMALIZE_SCORES:
        q_len = (np.asarray(q)[:, :, 0] != 0).sum(axis=1).astype(np.float64)
        scores = scores / q_len[:, None]
    logits = scores / TEMPERATURE
    m = logits.max(axis=1, keepdims=True)
    logz = m[:, 0] + np.log(np.exp(logits - m).sum(axis=1))
    loss = -(np.diag(logits) - logz).mean()
    return np.float32(loss)


def kernel(query_embeddings, doc_embeddings):
    q = np.asarray(query_embeddings, dtype=np.float32)
    nc = _get_nc()
    in_maps = make_inmaps(q, doc_embeddings)
    res = run_bass_kernel_spmd(nc, in_maps, core_ids=list(range(NCORES)))
    outs = [res.results[k]["out"] for k in range(NCORES)]
    sums = [res.results[k]["sums"] for k in range(NCORES)]
    return assemble_loss(outs, sums, q)


# revision 23
# speedup vs baseline: 1.3306x; 1.3306x over previous
"""ColBERT MaxSim contrastive loss on 8 Trainium2 NeuronCores.

scores[b, c] = (1/q_len[b]) * sum_n max_s <q[b, n, :], d[c, s, :]>
loss = CE(scores / T, labels=arange(B)), mean reduction.

Sharding: data-parallel over the *doc* batch dim (columns of the score
matrix). Each core holds the full query set plus its 8-doc shard,
computes its (64, 8) score block, and the host performs the final
gather + tiny 64x64 CE reduction.

v2 design (trace-driven):
  * Host pre-transposes and fp16-casts q and d into [D, token] layout,
    so the device does plain contiguous DMAs (2-4 KB per partition) and
    starts the first matmul within a few us -- the v1 on-device
    load+cast+xbar-transpose prologue cost 28 us of PE idle.
  * Per (query-group g, doc) set: two fp16 matmuls -> [128, 1024] PSUM
    tile (4 rotating slots = all 8 banks). PE streams 512 cols / 215 ns.
  * PSUM max-drain, the v1 bottleneck (ACT 117us + DVE 108us busy),
    is split across both engines with single-instruction fused ops:
      - DVE docs: tensor_tensor_reduce(max, max) reading the two PSUM
        halves via both operand ports (2 elem/cyc) -> exact max in one
        ~557 ns op.
      - ACT docs: activation(Exp, scale=a, bias=-0.9a, accum_out=sum)
        drains 1024 elems at 1 elem/cyc; a single Log pass at the end
        turns the per-doc sums into smooth-maxes:
            max_s x ~= 0.9 + ln(sum_s e^{a(x_s-0.9)}) / a,  a = 512.
        Upward bias <= ln(1024)/a ~ 0.0135 worst case, ~0.002 typical,
        and mostly common-mode across score columns (cancels in
        softmax); measured end-to-end loss rel-err 6e-5.
  * A tiny selector matmul sums the 32 token-maxes per query:
    out[4, 128] = sel.T @ maxes. Host unscrambles + CE.
"""

import json

import numpy as np

import concourse.bass as bass
import concourse.mybir as mybir
import concourse.tile as tile
from concourse.bass_utils import run_bass_kernel_spmd

B = 64          # queries (= docs, contrastive batch)
NQ = 32         # tokens per query
ND = 1024       # tokens per doc
D = 128         # embedding dim
NCORES = 8
CL = B // NCORES  # docs per core
TEMPERATURE = 0.02
NORMALIZE_SCORES = True

F32 = mybir.dt.float32
F16 = mybir.dt.float16

NG = (B * NQ) // 128        # 16 query groups of 4 queries
NSETS = NG * CL             # 128 (query group, doc) sets

ALPHA = 512.0               # smooth-max sharpness
CENTER = 0.9                # exp recentering (sims in [0.55, 0.90])

# Per-doc drain engine assignment. DVE can read only ONE operand from
# PSUM per instruction (NCC_IBVF027), so both drain engines run at
# ~1 elem/cyc. Measured steady-state costs: DVE reduce_max 1215ns/doc
# (1035ns/doc when two adjacent PSUM slots batch into one [128,2,1024]
# reduce), ACT exp-accum 1113+283(READ_ACC) ns/doc. A 5/3 split with
# two batched DVE pairs balances at ~5.4us vs ~4.6us per group.
DVE_DOCS = (0, 2, 4, 6)     # exact reduce_max on DVE
ACT_DOCS = (1, 3, 5, 7)     # smooth-max exp-accum on ACT
MAX_EXP = len(ACT_DOCS)     # exp docs per group (sums tile width)
ACT_SLOT = {c: i for i, c in enumerate(ACT_DOCS)}


def _split_waits_json(bir_bytes: bytes) -> bytes:
    """Walrus in this toolchain rejects >1 sem-wait per instruction on the
    Tile end-of-kernel drain; split extra waits onto preceding Drains."""
    bir = json.loads(bir_bytes)
    for f in bir["functions"]:
        for blk in f["blocks"]:
            fixed = []
            for ins in blk["instructions"]:
                si = ins.get("sync_info") or {}
                waits = si.get("on_wait") or []
                if len(waits) > 1:
                    for i, w in enumerate(waits[:-1]):
                        fixed.append({
                            "debug": ins.get("debug", 0),
                            "engine": ins["engine"],
                            "ins": [],
                            "is_reset_sema": False,
                            "name": f'{ins["name"]}-wsplit{i}',
                            "opcode": "Drain",
                            "outs": [],
                            "sync_info": {"on_update": [], "on_wait": [w]},
                        })
                    si["on_wait"] = waits[-1:]
                    ins["sync_info"] = si
                fixed.append(ins)
            blk["instructions"] = fixed
    return json.dumps(bir).encode()


def _patch_nc(nc):
    orig = nc.to_json_bytes

    def patched(*a, **k):
        return _split_waits_json(orig(*a, **k))

    nc.to_json_bytes = patched
    return nc


def build_nc():
    """Build the per-core Bass program (SPMD: every core runs this; only
    the data in its "dT" shard differs)."""
    nc = bass.Bass("TRN2", target_bir_lowering=False, debug=False,
                   num_devices=NCORES)
    qT_dram = nc.dram_tensor("qT", [D, B * NQ], F16, kind="ExternalInput").ap()
    dT_dram = nc.dram_tensor("dT", [D, CL * ND], F16,
                             kind="ExternalInput").ap()
    sel_dram = nc.dram_tensor("sel", [128, 4], F16, kind="ExternalInput").ap()
    out_dram = nc.dram_tensor("out", [4, NSETS], F32, kind="ExternalOutput").ap()
    sums_dram = nc.dram_tensor("sums", [128, NG * MAX_EXP], F32,
                               kind="ExternalOutput").ap()

    EXP = mybir.ActivationFunctionType.Exp

    with tile.TileContext(nc) as tc:
        with (
            tc.tile_pool(name="prep", bufs=1) as prep,
            tc.tile_pool(name="eo", bufs=2) as eo_pool,
            tc.tile_pool(name="mm", bufs=4, space="PSUM") as psum_pool,
        ):
            # ---- loads: host-pretransposed fp16 [D, token] tensors.
            # Split so the g=0 chain (qT group 0 + docs 0-1) lands first,
            # alternating issue queues (sync/scalar are the HWDGE
            # engines); each chunk is a contiguous 4 KB/partition. ----
            qT0 = prep.tile([128, 128], F16, tag="qT0", name="qT0")
            qTr = prep.tile([128, (NG - 1) * 128], F16, tag="qTr",
                            name="qTr")
            nc.sync.dma_start(qT0[:], qT_dram[:, 0:128])
            nc.scalar.dma_start(qTr[:], qT_dram[:, 128:NG * 128])
            dT = []   # 4 tiles of 2 docs each
            for p in range(CL // 2):
                t = prep.tile([128, 2 * ND], F16, tag=f"dT{p}",
                              name=f"dT{p}")
                eng = nc.sync if p % 2 == 0 else nc.scalar
                eng.dma_start(t[:], dT_dram[:, 2 * p * ND:(2 * p + 2) * ND])
                dT.append(t)
            sel = prep.tile([128, 4], F16)
            nc.sync.dma_start(sel[:], sel_dram)

            maxes = prep.tile([128, NSETS], F16)
            sums = prep.tile([128, NG * MAX_EXP], F32)
            # exp-doc columns of `maxes` are never written on-device (the
            # host takes them from `sums`); zero them so the selector
            # matmul can't see garbage/NaN.
            nc.vector.memset(maxes[:], 0.0)
            # warm the ACT exp table-set (~2.7us) during the DMA window
            # instead of on the first real exp-drain; exp_bias doubles as
            # the (AP-only) bias operand of the real exp-drains
            warm = prep.tile([128, 1], F32)
            nc.vector.memset(warm[:], 0.0)
            exp_bias = prep.tile([128, 1], F32)
            nc.vector.memset(exp_bias[:], -ALPHA * CENTER)
            nc.scalar.activation(warm[:], warm[:], EXP)

            # ---- main loop: 16 query groups x 8 docs, one doc per
            # [128, 1024] PSUM tile (4 rotating slots = all 8 banks) ----
            for g in range(NG):
                lhs = qT0 if g == 0 else qTr[:, bass.ts(g - 1, 128)]
                for doc in range(CL):
                    idx = g * CL + doc
                    rhs = dT[doc // 2][:, (doc % 2) * ND:(doc % 2 + 1) * ND]
                    pa = psum_pool.tile([128, ND], F32, tag="pa", name="pa")
                    nc.tensor.matmul(pa[:, 0:512], lhs, rhs[:, 0:512],
                                     start=True, stop=True)
                    nc.tensor.matmul(pa[:, 512:1024], lhs, rhs[:, 512:1024],
                                     start=True, stop=True)
                    if doc in DVE_DOCS:
                        # exact max, single-operand PSUM read on DVE
                        nc.vector.reduce_max(
                            maxes[:, idx:idx + 1], pa[:],
                            axis=mybir.AxisListType.X)
                    else:
                        # smooth-max: ACT exp-drain with fused sum
                        j = g * MAX_EXP + ACT_SLOT[doc]
                        eo = eo_pool.tile([128, ND], F32, tag="eo",
                                          name="eo")
                        nc.scalar.activation(
                            eo[:], pa[:], EXP,
                            bias=exp_bias[:], scale=ALPHA,
                            accum_out=sums[:, j:j + 1])
                if g == NG // 2 - 1:
                    # ship the first half of the exp sums early
                    nc.sync.dma_start(
                        sums_dram[:, 0:NG * MAX_EXP // 2],
                        sums[:, 0:NG * MAX_EXP // 2])

            # rest of the exp-doc sums; host computes the ln
            nc.sync.dma_start(sums_dram[:, NG * MAX_EXP // 2:],
                              sums[:, NG * MAX_EXP // 2:])

            # ---- sum the 32 tokens of each query: out[4, NSETS] ----
            sel_ps = psum_pool.tile([4, NSETS], F32, tag="pa", name="selps")
            nc.tensor.matmul(sel_ps[:], sel[:], maxes[:], start=True,
                             stop=True)
            out_sb = prep.tile([4, NSETS], F32)
            nc.vector.tensor_copy(out_sb[:], sel_ps[:])
            nc.sync.dma_start(out_dram, out_sb[:])

    nc.finalize()
    return _patch_nc(nc)


_NC = None


def _get_nc():
    global _NC
    if _NC is None:
        _NC = build_nc()
    return _NC


def make_sel():
    # sel[p, m] = 1 iff token-partition p belongs to query m of its group
    sel = np.zeros((128, 4), np.float16)
    for m in range(4):
        sel[NQ * m:NQ * (m + 1), m] = 1.0
    return sel


def make_inmaps(q, d):
    """Host prep: [D, token] fp16 layouts + per-core doc shards."""
    q = np.asarray(q, dtype=np.float32)
    d = np.asarray(d, dtype=np.float32)
    qT = np.ascontiguousarray(
        q.reshape(B * NQ, D).T.astype(np.float16))          # [128, 2048]
    dT_full = d.transpose(2, 0, 1).astype(np.float16)       # [128, 64, 1024]
    sel = make_sel()
    return [
        {"qT": qT,
         "dT": np.ascontiguousarray(
             dT_full[:, CL * k:CL * (k + 1)]).reshape(D, CL * ND),
         "sel": sel}
        for k in range(NCORES)
    ]


def assemble_loss(outs, sums, q):
    """Host tail: per-core [4, NSETS] + raw exp-sums -> scores -> CE.

    out[m, g*8+c] (DVE docs) = sum over the 32 tokens of query 4g+m of
    the exact token-max. For ACT exp docs the device ships S = sum_s
    e^{a(sim-0.9)} per (token-partition, g, e) and the host applies
    max ~= 0.9 + ln(S)/a and the 32-token sum itself."""
    scores = np.zeros((B, B), np.float64)
    for k in range(NCORES):
        blk = np.asarray(outs[k], np.float64).reshape(4, NG, CL)
        # token-sums of ln(S): [128, NG, MAX_EXP] -> [4, NG, MAX_EXP]
        lnS = np.log(np.asarray(sums[k], np.float64)).reshape(
            4, NQ, NG, MAX_EXP).sum(axis=1)
        for g in range(NG):
            for m in range(4):
                for c in range(CL):
                    if c in DVE_DOCS:
                        v = blk[m, g, c]
                    else:
                        v = NQ * CENTER + lnS[m, g, ACT_SLOT[c]] / ALPHA
                    scores[4 * g + m, CL * k + c] = v
    if NORMALIZE_SCORES:
        q_len = (np.asarray(q)[:, :, 0] != 0).sum(axis=1).astype(np.float64)
        scores = scores / q_len[:, None]
    logits = scores / TEMPERATURE
    m = logits.max(axis=1, keepdims=True)
    logz = m[:, 0] + np.log(np.exp(logits - m).sum(axis=1))
    loss = -(np.diag(logits) - logz).mean()
    return np.float32(loss)


def kernel(query_embeddings, doc_embeddings):
    q = np.asarray(query_embeddings, dtype=np.float32)
    nc = _get_nc()
    in_maps = make_inmaps(q, doc_embeddings)
    res = run_bass_kernel_spmd(nc, in_maps, core_ids=list(range(NCORES)))
    outs = [res.results[k]["out"] for k in range(NCORES)]
    sums = [res.results[k]["sums"] for k in range(NCORES)]
    return assemble_loss(outs, sums, q)
